# revision 1
# baseline (speedup 1.0000x reference)
"""Trainium2 Bass kernel for quantized 3x3 conv2d (stride 1, pad 1).

Reference computes: conv2d(quant16(x), quant16(w)) where quant16 rounds to
signed 16-bit fixed point with 12 fractional bits (round-half-even, /4096).

Strategy (per core, data-parallel over batch: 4 images/core on 8 cores):
  - Quantize on device with the magic-number trick (+1.5*2^23 in f32 RNE)
    giving rx = round(x*4096) exactly (round-half-even, matches jnp.round).
  - rx needs 16 bits; fp16 holds 11-bit mantissas, so split rx into two
    exact fp16 terms: Xh = fp16(rx) (RNE), Xl = rx - Xh (|Xl| <= 16).
    rw = round(w*4096) fits fp16 exactly (|rw| ~ 1100 < 2048).
  - 3x3 conv = 9 shifted matmuls accumulating in PSUM over a zero-padded
    58x58 image laid out [Cin=128 partitions, 58*58]. Contraction dim =
    partition dim = Cin = 128. Cout=256 -> two 128-row output chunks.
    2 fp16 terms x 9 taps x 2 Cout halves accumulate per output tile.
  - PSUM result = 2^24 * conv(qx, qw); the PSUM->SBUF eviction copy applies
    the 2^-24 scale for free (ScalarE activation Copy with scale).
  - Products are exact in fp32 (11x12-bit mantissas), so accuracy matches
    the f32 reference up to accumulation order.
"""

import numpy as np

B, CIN, COUT, H, W = 32, 128, 256, 56, 56
NCORES = 8
BL = B // NCORES          # images per core
HP = H + 2                # padded height/width (58)
NPIX = H * W              # 3136
NPAD = HP * HP            # 3364
SCALE = 4096.0
MAGIC = 12582912.0        # 1.5 * 2**23: f32 add forces round-to-nearest-even at ulp=1
OSCALE = 1.0 / (SCALE * SCALE)
GROUP_ROWS = 7            # output rows per PSUM tile
NGRP = H // GROUP_ROWS    # 8 groups of 392 px
GRP_PIX = GROUP_ROWS * W  # 392
ROUND_PIX = 4 * GRP_PIX   # 1568 px per PSUM round (4 banks)

_cache = {}


def _build():
    import concourse.bacc as bacc
    import concourse.mybir as mybir
    import concourse.tile as tile

    f32, f16 = mybir.dt.float32, mybir.dt.float16
    Copy = mybir.ActivationFunctionType.Copy
    Alu = mybir.AluOpType

    nc = bacc.Bacc("TRN2", target_bir_lowering=False)
    # x arrives zero-padded to 58x58 from the host so every DMA is contiguous
    x_in = nc.dram_tensor("x", [BL, CIN, NPAD], f32, kind="ExternalInput")
    w_in = nc.dram_tensor("w", [CIN, 9 * COUT], f32, kind="ExternalInput")
    out = nc.dram_tensor("out", [BL, COUT, NPIX], f32, kind="ExternalOutput")

    with tile.TileContext(nc) as tc:
        with (
            tc.tile_pool(name="fixed", bufs=1) as fx,
            tc.tile_pool(name="psum", bufs=1, space="PSUM") as pp,
        ):
            # ---- per-image ping-pong buffers ----
            xsts = [fx.tile([CIN, NPAD], f32, name=f"xst{i}") for i in range(2)]
            ts = [fx.tile([CIN, NPAD], f32, name=f"t{i}") for i in range(2)]
            xhs = [fx.tile([CIN, NPAD], f16, name=f"xh{i}") for i in range(2)]
            xh32s = [fx.tile([CIN, NPAD], f32, name=f"xh32_{i}") for i in range(2)]
            xls = [fx.tile([CIN, NPAD], f16, name=f"xl{i}") for i in range(2)]
            osbs = [fx.tile([128, ROUND_PIX], f32, name=f"osb{i}") for i in range(3)]
            ps = [pp.tile([128, GRP_PIX], f32, name=f"ps{i}") for i in range(8)]
            wst = fx.tile([CIN, 9 * COUT], f32)
            wt = fx.tile([CIN, 9 * COUT], f32)
            w16 = fx.tile([CIN, 9 * COUT], f16)

            # Staging is split into two row-chunks so the quantize chain (and
            # the first PE round) starts before the whole image has landed.
            # Chunk 0 = padded rows [0, 30) (everything PE rounds half=0 read),
            # chunk 1 = padded rows [30, 58).
            CHUNKS = [(0, 30), (30, HP)]

            def stage_chunk(b, c, rng=None):
                s = b % 2
                xst, t, xh, xh32, xl = xsts[s], ts[s], xhs[s], xh32s[s], xls[s]
                r0, r1 = rng if rng is not None else CHUNKS[c]
                lo, hi = r0 * HP, r1 * HP
                nc.sync.dma_start(out=xst[:, lo:hi], in_=x_in[b, :, lo:hi])
                # t = x*4096 + MAGIC  (exact fma; the add performs RNE rounding)
                nc.scalar.activation(t[:, lo:hi], xst[:, lo:hi], Copy, bias=MAGIC, scale=SCALE)
                # Xh = fp16(rx)  (f32 subtract exact, fp16 convert-on-write RNE)
                nc.vector.tensor_scalar_add(xh[:, lo:hi], t[:, lo:hi], -MAGIC)
                nc.scalar.activation(xh32[:, lo:hi], xh[:, lo:hi], Copy)
                # Xl = rx - Xh  (exact, |Xl| <= 16)
                nc.vector.scalar_tensor_tensor(
                    xl[:, lo:hi], t[:, lo:hi], -MAGIC, xh32[:, lo:hi],
                    Alu.add, Alu.subtract,
                )

            # ---- weights: load + quantize to fp16 integers (rw = round(w*4096)) ----
            # ch-major layout [ci, (ch, tap, co)]; the ch=0 half stages first
            # so the first LDWEIGHTS only waits for half the weight bytes.
            # Image-0 chunk-0 is issued first: its chain is the longest pole
            # to the first matmul.
            HW_COLS = 9 * 128  # 1152 columns per cout-half
            # image 0: a 10-row first slice so round-0's g=0 matmuls (rows <10)
            # start as early as possible
            stage_chunk(0, 0, rng=(0, 10))
            stage_chunk(0, 0, rng=(10, 30))
            for wc in range(2):
                lo, hi = wc * HW_COLS, (wc + 1) * HW_COLS
                nc.sync.dma_start(out=wst[:, lo:hi], in_=w_in[:, lo:hi])
                # rw+MAGIC then -MAGIC, both on DVE (two-op tensor_scalar)
                # to keep the ACT queue free for the image-0 chain
                nc.vector.tensor_scalar(
                    out=wt[:, lo:hi], in0=wst[:, lo:hi],
                    scalar1=SCALE, scalar2=MAGIC,
                    op0=Alu.mult, op1=Alu.add,
                )
                nc.vector.tensor_scalar_add(w16[:, lo:hi], wt[:, lo:hi], -MAGIC)
                if wc == 0:
                    stage_chunk(0, 1)
            stage_chunk(1, 0)
            stage_chunk(1, 1)

            rnd = 0
            for b in range(BL):
                if b >= 2:
                    stage_chunk(b, 0)
                    stage_chunk(b, 1)
                s = b % 2
                xh3 = xhs[s][:].rearrange("p (h w) -> p h w", h=HP)
                xl3 = xls[s][:].rearrange("p (h w) -> p h w", h=HP)

                for ch in range(2):
                    for half in range(2):
                        bank = (rnd % 2) * 4
                        osb = osbs[rnd % 3]
                        # First round: all-Xh taps first so the PE can start
                        # before Xl is staged. Steady state: taps outer so 8
                        # consecutive matmuls share one stationary weight.
                        if rnd == 0:
                            # g-major, Xh-first: g=0 only needs padded rows
                            # <10, so its matmuls start before the rest of
                            # the image (or Xl) has staged
                            seq0 = [(tap, term) for term in range(2) for tap in range(9)]
                            for g in range(4):
                                for si, (tap, term) in enumerate(seq0):
                                    dh, dw = divmod(tap, 3)
                                    wsl = w16[:, tap * 128 : tap * 128 + 128]
                                    xt3 = xh3 if term == 0 else xl3
                                    r0 = g * GROUP_ROWS
                                    mv = xt3[:, r0 + dh : r0 + dh + GROUP_ROWS, dw : dw + W]
                                    nc.tensor.matmul(
                                        ps[bank + g][:], wsl, mv,
                                        start=(si == 0), stop=(si == 17),
                                    )
                            seq = []
                        else:
                            seq = [(tap, term) for tap in range(9) for term in range(2)]
                        for si, (tap, term) in enumerate(seq):
                            dh, dw = divmod(tap, 3)
                            wsl = w16[:, ch * 1152 + tap * 128 : ch * 1152 + tap * 128 + 128]
                            xt3 = xh3 if term == 0 else xl3
                            for g in range(4):
                                r0 = (half * 4 + g) * GROUP_ROWS
                                mv = xt3[:, r0 + dh : r0 + dh + GROUP_ROWS, dw : dw + W]
                                nc.tensor.matmul(
                                    ps[bank + g][:],
                                    wsl,
                                    mv,
                                    start=(si == 0),
                                    stop=(si == 17),
                                )
                        last_round = rnd == BL * 4 - 1
                        if last_round:
                            # spread the tail: drains split ACT/DVE, per-bank
                            # stores so the final DMA isn't one serial lump
                            for g in range(4):
                                dst = osb[:, g * GRP_PIX : (g + 1) * GRP_PIX]
                                if g % 2 == 0:
                                    nc.scalar.activation(dst, ps[bank + g][:], Copy, scale=OSCALE)
                                else:
                                    nc.vector.tensor_scalar_mul(dst, ps[bank + g][:], OSCALE)
                                nc.sync.dma_start(
                                    out=out[
                                        b,
                                        ch * 128 : (ch + 1) * 128,
                                        half * ROUND_PIX + g * GRP_PIX : half * ROUND_PIX + (g + 1) * GRP_PIX,
                                    ],
                                    in_=dst,
                                )
                        else:
                            for g in range(4):
                                dst = osb[:, g * GRP_PIX : (g + 1) * GRP_PIX]
                                if g % 2 == 0:
                                    nc.scalar.activation(dst, ps[bank + g][:], Copy, scale=OSCALE)
                                else:
                                    nc.vector.tensor_scalar_mul(dst, ps[bank + g][:], OSCALE)
                            nc.sync.dma_start(
                                out=out[
                                    b,
                                    ch * 128 : (ch + 1) * 128,
                                    half * ROUND_PIX : (half + 1) * ROUND_PIX,
                                ],
                                in_=osb[:],
                            )
                        rnd += 1
    nc.compile()
    return nc


def _get_nc():
    if "nc" not in _cache:
        _cache["nc"] = _build()
    return _cache["nc"]


def _maybe_install_trace_bridge():
    """Optional: bridge antenv.axon_hooks so trace=True can capture NTFF."""
    import sys
    import types

    if "antenv.axon_hooks" in sys.modules:
        return
    try:
        from trn_agent_boot.trn_boot import _ntff_profile_via_ctypes

        hook = _ntff_profile_via_ctypes("/opt/axon/libaxon_pjrt.so")
        mod = types.ModuleType("antenv.axon_hooks")
        mod.get_axon_ntff_profile_hook = lambda: hook
        mod.set_axon_ntff_profile_hook = lambda h: None
        import antenv

        sys.modules["antenv.axon_hooks"] = mod
        antenv.axon_hooks = mod
    except Exception:
        pass


def kernel(**inputs):
    import os

    from concourse.bass_utils import run_bass_kernel_spmd

    x = np.ascontiguousarray(np.asarray(inputs["x"], dtype=np.float32))
    weight = np.ascontiguousarray(np.asarray(inputs["weight"], dtype=np.float32))
    assert x.shape == (B, CIN, H, W), x.shape
    assert weight.shape == (COUT, CIN, 3, 3), weight.shape

    # [Cout, Cin, kh, kw] -> [Cin, (ch, kh kw, co128)] so each (ch, tap)
    # slice is a ready [K=ci, M=co] stationary operand, ch-major so the
    # kernel can stage the ch=0 half first.
    w_r = np.ascontiguousarray(
        weight.reshape(2, 128, CIN, 9)
        .transpose(2, 0, 3, 1)
        .reshape(CIN, 9 * COUT)
    )
    xp = np.zeros((B, CIN, HP, HP), dtype=np.float32)
    xp[:, :, 1 : 1 + H, 1 : 1 + W] = x.reshape(B, CIN, H, W)
    xp = xp.reshape(B, CIN, NPAD)
    in_maps = [
        {"x": xp[i * BL : (i + 1) * BL], "w": w_r}
        for i in range(NCORES)
    ]

    trace = bool(int(os.environ.get("KERNEL_TRACE", "0")))
    if trace:
        _maybe_install_trace_bridge()
    nc = _get_nc()
    res = run_bass_kernel_spmd(nc, in_maps, core_ids=list(range(NCORES)), trace=trace)
    _cache["exec_time_ns"] = res.exec_time_ns
    _cache["res"] = res

    outs = [res.results[i]["out"].reshape(BL, COUT, H, W) for i in range(NCORES)]
    return np.concatenate(outs, axis=0)



# revision 2
# speedup vs baseline: 1.7540x; 1.7540x over previous
"""Trainium2 Bass kernel for quantized 3x3 conv2d (stride 1, pad 1).

Reference computes: conv2d(quant16(x), quant16(w)) where quant16 rounds to
signed 16-bit fixed point with 12 fractional bits (round-half-even, /4096).

Strategy (per core, data-parallel over batch: 4 images/core on 8 cores):
  - Quantize on device with the magic-number trick (+1.5*2^23 in f32 RNE)
    giving rx = round(x*4096) exactly (round-half-even, matches jnp.round).
  - Single fp16 term: Xh = fp16(rx) (RNE). |rx| <= 32768 so the fp16
    rounding error is <= 16 integer ulps (~2^-11 relative), which lands the
    conv output at ~2e-4 max rel err -- far inside the 2e-2 gate. The
    second correction term (Xl) costs a full extra matmul pass and is not
    needed at this tolerance. rw = round(w*4096) fits fp16 exactly
    (|rw| ~ 1100 < 2048).
  - 3x3 conv = 9 shifted matmuls accumulating in PSUM over a zero-padded
    58x58 image laid out [Cin=128 partitions, 58*58]. Contraction dim =
    partition dim = Cin = 128. Cout=256 -> two 128-row output chunks.
  - One round = (image, cout-chunk): 9 taps x 8 PSUM banks (all 56 output
    rows). Taps outer so 8 consecutive matmuls share one stationary weight.
  - PSUM result = 2^24 * conv(qx, qw); the PSUM->SBUF eviction copy applies
    the 2^-24 scale for free (ScalarE activation Copy with scale).
"""

import numpy as np

B, CIN, COUT, H, W = 32, 128, 256, 56, 56
NCORES = 8
BL = B // NCORES          # images per core
HP = H + 2                # padded height/width (58)
NPIX = H * W              # 3136
NPAD = HP * HP            # 3364
SCALE = 4096.0
MAGIC = 12582912.0        # 1.5 * 2**23: f32 add forces round-to-nearest-even at ulp=1
OSCALE = 1.0 / (SCALE * SCALE)
GROUP_ROWS = 7            # output rows per PSUM tile
NGRP = H // GROUP_ROWS    # 8 groups of 392 px
GRP_PIX = GROUP_ROWS * W  # 392

_cache = {}


def _build():
    import concourse.bacc as bacc
    import concourse.mybir as mybir
    import concourse.tile as tile

    f32, f16 = mybir.dt.float32, mybir.dt.float16
    Copy = mybir.ActivationFunctionType.Copy
    Alu = mybir.AluOpType

    nc = bacc.Bacc("TRN2", target_bir_lowering=False)
    # x arrives zero-padded to 58x58 from the host so every DMA is contiguous
    x_in = nc.dram_tensor("x", [BL, CIN, NPAD], f32, kind="ExternalInput")
    w_in = nc.dram_tensor("w", [CIN, 9 * COUT], f32, kind="ExternalInput")
    out = nc.dram_tensor("out", [BL, COUT, NPIX], f32, kind="ExternalOutput")

    with tile.TileContext(nc) as tc:
        with (
            tc.tile_pool(name="fixed", bufs=1) as fx,
            tc.tile_pool(name="psum", bufs=1, space="PSUM") as pp,
        ):
            # ---- per-image ping-pong buffers ----
            xsts = [fx.tile([CIN, NPAD], f32, name=f"xst{i}") for i in range(2)]
            ts = [fx.tile([CIN, NPAD], f32, name=f"t{i}") for i in range(2)]
            xhs = [fx.tile([CIN, NPAD], f16, name=f"xh{i}") for i in range(2)]
            osbs = [fx.tile([128, NPIX], f32, name=f"osb{i}") for i in range(3)]
            ps = [pp.tile([128, GRP_PIX], f32, name=f"ps{i}") for i in range(8)]
            wst = fx.tile([CIN, 9 * COUT], f32)
            wt = fx.tile([CIN, 9 * COUT], f32)
            w16 = fx.tile([CIN, 9 * COUT], f16)

            # Staging is split into row-chunks so the quantize chain (and the
            # first PE matmuls) start before the whole image has landed.
            CHUNKS = [(0, 30), (30, HP)]

            def stage_chunk(b, c, rng=None):
                s = b % 2
                xst, t, xh = xsts[s], ts[s], xhs[s]
                r0, r1 = rng if rng is not None else CHUNKS[c]
                lo, hi = r0 * HP, r1 * HP
                nc.sync.dma_start(out=xst[:, lo:hi], in_=x_in[b, :, lo:hi])
                # t = x*4096 + MAGIC  (exact fma; the add performs RNE rounding)
                nc.scalar.activation(t[:, lo:hi], xst[:, lo:hi], Copy, bias=MAGIC, scale=SCALE)
                # Xh = fp16(rx)  (f32 subtract exact, fp16 convert-on-write RNE)
                nc.vector.tensor_scalar_add(xh[:, lo:hi], t[:, lo:hi], -MAGIC)

            # ---- weights: load + quantize to fp16 integers (rw = round(w*4096)) ----
            # ch-major layout [ci, (ch, tap, co)]; the ch=0 half stages first
            # so the first LDWEIGHTS only waits for half the weight bytes.
            # Image-0 chunk-0 is issued first: its chain is the longest pole
            # to the first matmul.
            HW_COLS = 9 * 128  # 1152 columns per cout-half
            # image 0: a 17-row first slice so round-0's first group-pair
            # (rows <17) starts as early as possible
            stage_chunk(0, 0, rng=(0, 17))
            stage_chunk(0, 0, rng=(17, 30))
            for wc in range(2):
                lo, hi = wc * HW_COLS, (wc + 1) * HW_COLS
                nc.sync.dma_start(out=wst[:, lo:hi], in_=w_in[:, lo:hi])
                # rw+MAGIC then -MAGIC, both on DVE (two-op tensor_scalar)
                # to keep the ACT queue free for the image-0 chain
                nc.vector.tensor_scalar(
                    out=wt[:, lo:hi], in0=wst[:, lo:hi],
                    scalar1=SCALE, scalar2=MAGIC,
                    op0=Alu.mult, op1=Alu.add,
                )
                nc.vector.tensor_scalar_add(w16[:, lo:hi], wt[:, lo:hi], -MAGIC)
                if wc == 0:
                    stage_chunk(0, 1)
            stage_chunk(1, 0)
            stage_chunk(1, 1)

            NRND = BL * 2
            for b in range(BL):
                if b >= 2:
                    stage_chunk(b, 0)
                    stage_chunk(b, 1)
                s = b % 2
                xh3 = xhs[s][:].rearrange("p (h w) -> p h w", h=HP)

                for ch in range(2):
                    rnd = b * 2 + ch
                    osb = osbs[rnd % 3]

                    def mm(tap, g, si):
                        dh, dw = divmod(tap, 3)
                        wsl = w16[:, ch * HW_COLS + tap * 128 : ch * HW_COLS + tap * 128 + 128]
                        r0 = g * GROUP_ROWS
                        mv = xh3[:, r0 + dh : r0 + dh + GROUP_ROWS, dw : dw + W]
                        nc.tensor.matmul(
                            ps[g][:], wsl, mv,
                            start=(si == 0), stop=(si == 8),
                        )

                    if rnd == 0:
                        # group-pair-major: the first pair only needs padded
                        # rows <17, so its matmuls start before the rest of
                        # the image has staged. 2 matmuls per weight load.
                        for gp in range(4):
                            for tap in range(9):
                                for g in (2 * gp, 2 * gp + 1):
                                    mm(tap, g, tap)
                    else:
                        # steady state: taps outer -> 8 consecutive matmuls
                        # share one stationary weight
                        for tap in range(9):
                            for g in range(8):
                                mm(tap, g, tap)

                    if rnd == NRND - 1:
                        # spread the tail: drains split ACT/DVE, per-bank
                        # stores so the final DMA isn't one serial lump
                        for g in range(8):
                            dst = osb[:, g * GRP_PIX : (g + 1) * GRP_PIX]
                            if g % 2 == 0:
                                nc.scalar.activation(dst, ps[g][:], Copy, scale=OSCALE)
                            else:
                                nc.vector.tensor_scalar_mul(dst, ps[g][:], OSCALE)
                            nc.sync.dma_start(
                                out=out[
                                    b,
                                    ch * 128 : (ch + 1) * 128,
                                    g * GRP_PIX : (g + 1) * GRP_PIX,
                                ],
                                in_=dst,
                            )
                    else:
                        for g in range(8):
                            dst = osb[:, g * GRP_PIX : (g + 1) * GRP_PIX]
                            if g % 2 == 0:
                                nc.scalar.activation(dst, ps[g][:], Copy, scale=OSCALE)
                            else:
                                nc.vector.tensor_scalar_mul(dst, ps[g][:], OSCALE)
                        nc.sync.dma_start(
                            out=out[b, ch * 128 : (ch + 1) * 128, :],
                            in_=osb[:],
                        )
    nc.compile()
    return nc


def _get_nc():
    if "nc" not in _cache:
        _cache["nc"] = _build()
    return _cache["nc"]


def _maybe_install_trace_bridge():
    """Optional: bridge antenv.axon_hooks so trace=True can capture NTFF."""
    import sys
    import types

    if "antenv.axon_hooks" in sys.modules:
        return
    try:
        from trn_agent_boot.trn_boot import _ntff_profile_via_ctypes

        hook = _ntff_profile_via_ctypes("/opt/axon/libaxon_pjrt.so")
        mod = types.ModuleType("antenv.axon_hooks")
        mod.get_axon_ntff_profile_hook = lambda: hook
        mod.set_axon_ntff_profile_hook = lambda h: None
        import antenv

        sys.modules["antenv.axon_hooks"] = mod
        antenv.axon_hooks = mod
    except Exception:
        pass


def kernel(**inputs):
    import os

    from concourse.bass_utils import run_bass_kernel_spmd

    x = np.ascontiguousarray(np.asarray(inputs["x"], dtype=np.float32))
    weight = np.ascontiguousarray(np.asarray(inputs["weight"], dtype=np.float32))
    assert x.shape == (B, CIN, H, W), x.shape
    assert weight.shape == (COUT, CIN, 3, 3), weight.shape

    # [Cout, Cin, kh, kw] -> [Cin, (ch, kh kw, co128)] so each (ch, tap)
    # slice is a ready [K=ci, M=co] stationary operand, ch-major so the
    # kernel can stage the ch=0 half first.
    w_r = np.ascontiguousarray(
        weight.reshape(2, 128, CIN, 9)
        .transpose(2, 0, 3, 1)
        .reshape(CIN, 9 * COUT)
    )
    xp = np.zeros((B, CIN, HP, HP), dtype=np.float32)
    xp[:, :, 1 : 1 + H, 1 : 1 + W] = x.reshape(B, CIN, H, W)
    xp = xp.reshape(B, CIN, NPAD)
    in_maps = [
        {"x": xp[i * BL : (i + 1) * BL], "w": w_r}
        for i in range(NCORES)
    ]

    trace = bool(int(os.environ.get("KERNEL_TRACE", "0")))
    if trace:
        _maybe_install_trace_bridge()
    nc = _get_nc()
    res = run_bass_kernel_spmd(nc, in_maps, core_ids=list(range(NCORES)), trace=trace)
    _cache["exec_time_ns"] = res.exec_time_ns
    _cache["res"] = res

    outs = [res.results[i]["out"].reshape(BL, COUT, H, W) for i in range(NCORES)]
    return np.concatenate(outs, axis=0)


# revision 4
# speedup vs baseline: 1.8532x; 1.0566x over previous
"""Trainium2 Bass kernel for quantized 3x3 conv2d (stride 1, pad 1).

Reference computes: conv2d(quant16(x), quant16(w)) where quant16 rounds to
signed 16-bit fixed point with 12 fractional bits (round-half-even, /4096).

Strategy (per core, data-parallel over batch: 4 images/core on 8 cores):
  - Quantize on device with the magic-number trick (+1.5*2^23 in f32 RNE)
    giving rx = round(x*4096) exactly (round-half-even, matches jnp.round).
  - Single fp16 term: Xh = fp16(rx) (RNE). |rx| <= 32768 so the fp16
    rounding error is <= 16 integer ulps (~2^-11 relative), which lands the
    conv output at ~2e-4 max rel err -- far inside the 2e-2 gate. The
    second correction term (Xl) costs a full extra matmul pass and is not
    needed at this tolerance. rw = round(w*4096) fits fp16 exactly
    (|rw| ~ 1100 < 2048).
  - 3x3 conv = 9 shifted matmuls accumulating in PSUM over a zero-padded
    58x58 image laid out [Cin=128 partitions, 58*58]. Contraction dim =
    partition dim = Cin = 128. Cout=256 -> two 128-row output chunks.
  - One round = (image, cout-chunk): 9 taps x 8 PSUM banks (all 56 output
    rows). Taps outer so 8 consecutive matmuls share one stationary weight.
  - PSUM result = 2^24 * conv(qx, qw); the PSUM->SBUF eviction copy applies
    the 2^-24 scale for free (ScalarE activation Copy with scale).
"""

import numpy as np

B, CIN, COUT, H, W = 32, 128, 256, 56, 56
NCORES = 8
BL = B // NCORES          # images per core
HP = H + 2                # padded height/width (58)
NPIX = H * W              # 3136
NPAD = HP * HP            # 3364
SCALE = 4096.0
MAGIC = 12582912.0        # 1.5 * 2**23: f32 add forces round-to-nearest-even at ulp=1
OSCALE = 1.0 / (SCALE * SCALE)
GROUP_ROWS = 7            # output rows per PSUM tile
NGRP = H // GROUP_ROWS    # 8 groups of 392 px
GRP_PIX = GROUP_ROWS * W  # 392

_cache = {}


def _build():
    import concourse.bacc as bacc
    import concourse.mybir as mybir
    import concourse.tile as tile

    f32, f16 = mybir.dt.float32, mybir.dt.float16
    Copy = mybir.ActivationFunctionType.Copy
    Alu = mybir.AluOpType

    nc = bacc.Bacc("TRN2", target_bir_lowering=False)
    # x arrives zero-padded to 58x58 from the host so every DMA is contiguous
    x_in = nc.dram_tensor("x", [BL, CIN, NPAD], f32, kind="ExternalInput")
    w_in = nc.dram_tensor("w", [CIN, 9 * COUT], f32, kind="ExternalInput")
    out = nc.dram_tensor("out", [BL, COUT, NPIX], f32, kind="ExternalOutput")

    with tile.TileContext(nc) as tc:
        with (
            tc.tile_pool(name="fixed", bufs=1) as fx,
            tc.tile_pool(name="psum", bufs=1, space="PSUM") as pp,
        ):
            # ---- per-image ping-pong buffers ----
            xsts = [fx.tile([CIN, NPAD], f32, name=f"xst{i}") for i in range(2)]
            ts = [fx.tile([CIN, NPAD], f32, name=f"t{i}") for i in range(2)]
            xhs = [fx.tile([CIN, NPAD], f16, name=f"xh{i}") for i in range(2)]
            osbs = [fx.tile([128, NPIX], f32, name=f"osb{i}") for i in range(3)]
            ps = [pp.tile([128, GRP_PIX], f32, name=f"ps{i}") for i in range(8)]
            wst = fx.tile([CIN, 9 * COUT], f32)
            wt = fx.tile([CIN, 9 * COUT], f32)
            w16 = fx.tile([CIN, 9 * COUT], f16)

            # Staging is split into row-chunks so the quantize chain (and the
            # first PE matmuls) start before the whole image has landed.
            # x DMAs issue from the (otherwise idle) GpSimd queue so they
            # don't serialize behind the weight DMAs on the Sync queue.
            CHUNKS = [(0, 30), (30, HP)]

            def stage_chunk(b, c, rng=None):
                s = b % 2
                xst, t, xh = xsts[s], ts[s], xhs[s]
                r0, r1 = rng if rng is not None else CHUNKS[c]
                lo, hi = r0 * HP, r1 * HP
                nc.gpsimd.dma_start(out=xst[:, lo:hi], in_=x_in[b, :, lo:hi])
                # t = x*4096 + MAGIC  (exact fma; the add performs RNE rounding)
                nc.scalar.activation(t[:, lo:hi], xst[:, lo:hi], Copy, bias=MAGIC, scale=SCALE)
                # Xh = fp16(rx)  (f32 subtract exact, fp16 convert-on-write RNE)
                nc.vector.tensor_scalar_add(xh[:, lo:hi], t[:, lo:hi], -MAGIC)

            # ---- weights: load + quantize to fp16 integers (rw = round(w*4096)) ----
            # ch-major layout [ci, (ch, tap, co)]. Weight slices stage FIRST
            # (they gate the first LDWEIGHTS) in three pieces: ch0 taps 0-2,
            # ch0 taps 3-8, ch1 -- so w16 for the first matmuls is ready
            # almost as soon as the preamble ends.
            HW_COLS = 9 * 128  # 1152 columns per cout-half
            WSLICES = [(0, 384), (384, HW_COLS), (HW_COLS, 2 * HW_COLS)]

            def stage_w(lo, hi):
                nc.sync.dma_start(out=wst[:, lo:hi], in_=w_in[:, lo:hi])
                # rw+MAGIC then -MAGIC, both on DVE (two-op tensor_scalar)
                # to keep the ACT queue free for the image-0 chain
                nc.vector.tensor_scalar(
                    out=wt[:, lo:hi], in0=wst[:, lo:hi],
                    scalar1=SCALE, scalar2=MAGIC,
                    op0=Alu.mult, op1=Alu.add,
                )
                nc.vector.tensor_scalar_add(w16[:, lo:hi], wt[:, lo:hi], -MAGIC)

            stage_w(*WSLICES[0])
            # image 0 in four slices; round-0 is g-major so group g only
            # needs padded rows < 7g+9
            stage_chunk(0, 0, rng=(0, 9))
            # PE warmup: the tensor engine ramps from ~2x-slow to full speed
            # over ~3us of continuous execution. Run throwaway matmuls on the
            # already-staged weight tile while x is still landing so the ramp
            # happens off the critical path. ps[7]'s first real write
            # (start=True) is WAW-ordered after these on the same queue.
            for _ in range(8):
                nc.tensor.matmul(
                    ps[7][:, 0:384], w16[:, 0:128], w16[:, 0:384],
                    start=True, stop=True,
                )
            stage_chunk(0, 0, rng=(9, 17))
            stage_w(*WSLICES[1])
            stage_chunk(0, 0, rng=(17, 30))
            stage_w(*WSLICES[2])
            stage_chunk(0, 1)
            stage_chunk(1, 0)
            stage_chunk(1, 1)

            NRND = BL * 2
            for b in range(BL):
                if b >= 2:
                    stage_chunk(b, 0)
                    stage_chunk(b, 1)
                s = b % 2
                xh3 = xhs[s][:].rearrange("p (h w) -> p h w", h=HP)

                for ch in range(2):
                    rnd = b * 2 + ch
                    osb = osbs[rnd % 3]

                    def mm(tap, g, si):
                        dh, dw = divmod(tap, 3)
                        wsl = w16[:, ch * HW_COLS + tap * 128 : ch * HW_COLS + tap * 128 + 128]
                        r0 = g * GROUP_ROWS
                        mv = xh3[:, r0 + dh : r0 + dh + GROUP_ROWS, dw : dw + W]
                        nc.tensor.matmul(
                            ps[g][:], wsl, mv,
                            start=(si == 0), stop=(si == 8),
                        )

                    if rnd == 0 or rnd == NRND - 1:
                        # g-major. Round 0: group g only needs padded rows
                        # <7g+9, so matmuls start while the image stages.
                        # Last round: each bank finishes 9 taps early, so its
                        # evict+store overlaps the remaining banks' matmuls
                        # and the tail after the final matmul is one bank.
                        for g in range(8):
                            for tap in range(9):
                                mm(tap, g, tap)
                    else:
                        # steady state: taps outer -> 8 consecutive matmuls
                        # share one stationary weight
                        for tap in range(9):
                            for g in range(8):
                                mm(tap, g, tap)

                    if rnd == NRND - 1:
                        # per-bank evict (split ACT/DVE) + per-bank store,
                        # DMA issues alternating Sync/GpSimd so the in-order
                        # issue cost doesn't serialize the tail
                        for g in range(8):
                            dst = osb[:, g * GRP_PIX : (g + 1) * GRP_PIX]
                            if g % 2 == 0:
                                nc.scalar.activation(dst, ps[g][:], Copy, scale=OSCALE)
                            else:
                                nc.vector.tensor_scalar_mul(dst, ps[g][:], OSCALE)
                            eng = nc.sync if g % 2 == 0 else nc.gpsimd
                            eng.dma_start(
                                out=out[
                                    b,
                                    ch * 128 : (ch + 1) * 128,
                                    g * GRP_PIX : (g + 1) * GRP_PIX,
                                ],
                                in_=dst,
                            )
                    else:
                        for g in range(8):
                            dst = osb[:, g * GRP_PIX : (g + 1) * GRP_PIX]
                            if g % 2 == 0:
                                nc.scalar.activation(dst, ps[g][:], Copy, scale=OSCALE)
                            else:
                                nc.vector.tensor_scalar_mul(dst, ps[g][:], OSCALE)
                        nc.sync.dma_start(
                            out=out[b, ch * 128 : (ch + 1) * 128, :],
                            in_=osb[:],
                        )
    nc.compile()
    return nc


def _get_nc():
    if "nc" not in _cache:
        _cache["nc"] = _build()
    return _cache["nc"]


def _maybe_install_trace_bridge():
    """Optional: bridge antenv.axon_hooks so trace=True can capture NTFF."""
    import sys
    import types

    if "antenv.axon_hooks" in sys.modules:
        return
    try:
        from trn_agent_boot.trn_boot import _ntff_profile_via_ctypes

        hook = _ntff_profile_via_ctypes("/opt/axon/libaxon_pjrt.so")
        mod = types.ModuleType("antenv.axon_hooks")
        mod.get_axon_ntff_profile_hook = lambda: hook
        mod.set_axon_ntff_profile_hook = lambda h: None
        import antenv

        sys.modules["antenv.axon_hooks"] = mod
        antenv.axon_hooks = mod
    except Exception:
        pass


def kernel(**inputs):
    import os

    from concourse.bass_utils import run_bass_kernel_spmd

    x = np.ascontiguousarray(np.asarray(inputs["x"], dtype=np.float32))
    weight = np.ascontiguousarray(np.asarray(inputs["weight"], dtype=np.float32))
    assert x.shape == (B, CIN, H, W), x.shape
    assert weight.shape == (COUT, CIN, 3, 3), weight.shape

    # [Cout, Cin, kh, kw] -> [Cin, (ch, kh kw, co128)] so each (ch, tap)
    # slice is a ready [K=ci, M=co] stationary operand, ch-major so the
    # kernel can stage the ch=0 half first.
    w_r = np.ascontiguousarray(
        weight.reshape(2, 128, CIN, 9)
        .transpose(2, 0, 3, 1)
        .reshape(CIN, 9 * COUT)
    )
    xp = np.zeros((B, CIN, HP, HP), dtype=np.float32)
    xp[:, :, 1 : 1 + H, 1 : 1 + W] = x.reshape(B, CIN, H, W)
    xp = xp.reshape(B, CIN, NPAD)
    in_maps = [
        {"x": xp[i * BL : (i + 1) * BL], "w": w_r}
        for i in range(NCORES)
    ]

    trace = bool(int(os.environ.get("KERNEL_TRACE", "0")))
    if trace:
        _maybe_install_trace_bridge()
    nc = _get_nc()
    res = run_bass_kernel_spmd(nc, in_maps, core_ids=list(range(NCORES)), trace=trace)
    _cache["exec_time_ns"] = res.exec_time_ns
    _cache["res"] = res

    outs = [res.results[i]["out"].reshape(BL, COUT, H, W) for i in range(NCORES)]
    return np.concatenate(outs, axis=0)
